# revision 1
# baseline (speedup 1.0000x reference)
"""Causal multi-head attention (B=4, S=2048, D=1024, H=16) on 8 TRN2 NeuronCores.

Sharding: core = (batch b, head-group g); g in {0,1} covers 8 heads (512 dims).
Each core computes its batch's attention for its 8 heads plus the partial
output projection; the host sums the two per-batch partials.

Device algorithm per core:
  - Q/K projections in bf16 hi/lo split precision (3-term products give
    ~16-bit effective mantissa); rope'd Q/K stored as fp32r hi/lo pairs
    (fp32r = RNE at 11 mantissa bits, measured; the PE multiplies
    pre-rounded operands exactly, so 3-term fp32r products reproduce
    full-fp32 scores). V single-term bf16.
  - RoPE rewritten in rotate-half form via a host-side feature permutation of
    q_w/k_w rows (scores are invariant); the half-swap is a SBUF->SBUF DMA
    (cross-partition), the cos/sin combine runs on Pool+DVE.
  - Rope'd Q/K roundtrip through internal DRAM to re-layout pair-tiles into
    per-head tiles with a K=65 augmented contraction: lhsT = [K_h; ones],
    rhs = [Q_h; -rowmax], so the exp shift costs no extra instruction.
  - Pass 1 computes scores in [q, kv] orientation for the row max (DVE
    reduce_max, negate=True); pass 2 computes scores transposed [kv, q]
    (K=65 fold applies the shift) and ACT exp writes bf16 attn^T directly
    to SBUF -- no on-chip transposes of the attention matrix.
  - AV contracts kv with V augmented by a ones column: PSUM row 64 is the
    softmax denominator; normalization = DVE reciprocal + rank-1 matmul
    replication (gpsimd partition_broadcast reads physical partition 0 on
    HW, diverging from CoreSim -- avoided) + one DVE multiply per
    (head, q-512 block).
  - Output projection contracts head pairs (K=128) with normalization
    already applied; odd heads reach partition rows 64..127 of the pair
    tiles via small SBUF->SBUF DMAs (engines cannot move partitions).
"""

import numpy as np

P = 128
B, S_FULL, DM = 4, 2048, 1024
H, DK = 16, 64
NG = 2
HPG = 8
DG = 512
THETA = 10000.0
MASK_VAL = -1e9


BUILD_PHASES = 2


def build_nc(S):
    import concourse.bacc as bacc
    import concourse.mybir as mybir
    import concourse.tile as tile

    dt = mybir.dt
    ST = S // P           # 128-row s tiles
    NJ = ST // 4          # q-512 groups
    SC = S // 512         # 512-wide column chunks

    TC = tile.TileContext
    nc = bacc.Bacc(None, target_bir_lowering=False)

    xTh = nc.dram_tensor("xTh", [DM, S], dt.bfloat16, kind="ExternalInput")
    xTl = nc.dram_tensor("xTl", [DM, S], dt.bfloat16, kind="ExternalInput")
    qkwh = nc.dram_tensor("qkwh", [DM, DM], dt.bfloat16, kind="ExternalInput")
    qkwl = nc.dram_tensor("qkwl", [DM, DM], dt.bfloat16, kind="ExternalInput")
    vw = nc.dram_tensor("vw", [DM, DG], dt.bfloat16, kind="ExternalInput")
    ow = nc.dram_tensor("ow", [DG, DM], dt.bfloat16, kind="ExternalInput")
    cs = nc.dram_tensor("cs", [P, 2, S], dt.float32, kind="ExternalInput")
    idr = nc.dram_tensor("idr", [P, P], dt.float32r, kind="ExternalInput")
    on1 = nc.dram_tensor("on1", [P, DK], dt.float32r, kind="ExternalInput")
    mkm = nc.dram_tensor("mkm", [P, P], dt.float32r, kind="ExternalInput")
    md = nc.dram_tensor("md", [P, 4, 512], dt.float32r, kind="ExternalInput")
    opart = nc.dram_tensor("opart", [S, DM], dt.float32, kind="ExternalOutput")
    qkrot = nc.dram_tensor("qkrot", [2, 2, HPG, DK, S], dt.float32r, kind="Internal")

    AluOp = mybir.AluOpType
    Act = mybir.ActivationFunctionType

    with TC(nc) as tc:
        with (
            tc.tile_pool(name="cp", bufs=1) as cp,
            tc.tile_pool(name="ps", bufs=1, space="PSUM") as ps,
        ):
            idrsb = cp.tile([P, P], dt.float32r, tag="idr", name="idrsb")
            nc.gpsimd.dma_start(idrsb[:], idr[:])
            on1sb = cp.tile([P, DK], dt.float32r, tag="on1", name="on1sb")
            nc.gpsimd.dma_start(on1sb[:], on1[:])
            mkmsb = cp.tile([P, P], dt.float32r, tag="mkm", name="mkmsb")
            nc.gpsimd.dma_start(mkmsb[:], mkm[:])
            vaug = cp.tile([P, ST, HPG, DK + 1], dt.bfloat16, tag="vaug", name="vaug")
            nc.gpsimd.memset(vaug[:, :, :, DK : DK + 1], 1.0)
            aoT = []
            for pr in range(4):
                t_ = cp.tile([P, S], dt.bfloat16, tag=f"aoT{pr}", name=f"aoT{pr}")
                aoT.append(t_)

            # ---------------- phase 1: projections + rope ----------------
            with tc.tile_pool(name="xp", bufs=1) as xp:
                xsb = xp.tile([P, 8, S], dt.bfloat16, tag="xsb", name="xsb")
                nc.gpsimd.dma_start(xsb[:], xTh.rearrange("(kt p) s -> p kt s", p=P))
                xsl = xp.tile([P, 8, S], dt.bfloat16, tag="xsl", name="xsl")
                nc.gpsimd.dma_start(xsl[:], xTl.rearrange("(kt p) s -> p kt s", p=P))

                with tc.tile_pool(name="vp", bufs=1) as vp:
                    vwsb = vp.tile([P, 8, DG], dt.bfloat16, tag="vwsb", name="vwsb")
                    nc.gpsimd.dma_start(
                        vwsb[:], vw.rearrange("(kt p) o -> p kt o", p=P)
                    )
                    for st in range(ST):
                        vps = ps.tile(
                            [P, DG], dt.float32, tag="pj", bufs=2, name=f"vps{st}"
                        )
                        for k in range(8):
                            nc.tensor.matmul(
                                vps[:],
                                lhsT=xsb[:, k, st * P : (st + 1) * P],
                                rhs=vwsb[:, k, :],
                                start=(k == 0),
                                stop=(k == 7),
                            )
                        nc.scalar.copy(
                            vaug[:, st, :, 0:DK],
                            vps[:].rearrange("p (h d) -> p h d", d=DK),
                        )

                with tc.tile_pool(name="qk", bufs=1) as qk:
                    cssb = qk.tile([P, 2, S], dt.float32, tag="cssb", name="cssb")
                    nc.gpsimd.dma_start(cssb[:], cs[:])
                    qkws = []
                    for t in range(2):
                        qkwsb = qk.tile(
                            [P, 8, DG], dt.bfloat16, tag="qkwsb", bufs=2,
                            name=f"qkwsb{t}",
                        )
                        nc.gpsimd.dma_start(
                            qkwsb[:],
                            qkwh[:, t * DG : (t + 1) * DG].rearrange(
                                "(kt p) o -> p kt o", p=P
                            ),
                        )
                        qkwsl = qk.tile(
                            [P, 8, DG], dt.bfloat16, tag="qkwsl", bufs=2,
                            name=f"qkwsl{t}",
                        )
                        nc.gpsimd.dma_start(
                            qkwsl[:],
                            qkwl[:, t * DG : (t + 1) * DG].rearrange(
                                "(kt p) o -> p kt o", p=P
                            ),
                        )
                        qkws.append((qkwsb, qkwsl))
                    for pr in range(4):
                        for t in range(2):
                            qkwsb, qkwsl = qkws[t]
                            qraw = qk.tile(
                                [P, S], dt.float32, tag="qraw", bufs=2,
                                name=f"qraw{t}{pr}",
                            )
                            for ch in range(SC):
                                qps = ps.tile(
                                    [P, 512], dt.float32, tag="pj", bufs=2,
                                    name=f"qps{t}{pr}{ch}",
                                )
                                terms = [
                                    (qkwsb, xsb), (qkwsb, xsl), (qkwsl, xsb)
                                ]
                                for ti, (wt, xt) in enumerate(terms):
                                    for k in range(8):
                                        nc.tensor.matmul(
                                            qps[:],
                                            lhsT=wt[:, k, pr * P : (pr + 1) * P],
                                            rhs=xt[:, k, ch * 512 : (ch + 1) * 512],
                                            start=(ti == 0 and k == 0),
                                            stop=(ti == 2 and k == 7),
                                        )
                                nc.scalar.copy(
                                    qraw[:, ch * 512 : (ch + 1) * 512], qps[:]
                                )
                            # rotate-half swap: SBUF->SBUF DMA partition move
                            qsw = qk.tile(
                                [P, S], dt.float32, tag="qsw", bufs=2,
                                name=f"qsw{t}{pr}",
                            )
                            nc.gpsimd.dma_start(qsw[0:32, :], qraw[32:64, :])
                            nc.gpsimd.dma_start(qsw[32:64, :], qraw[0:32, :])
                            nc.gpsimd.dma_start(qsw[64:96, :], qraw[96:128, :])
                            nc.gpsimd.dma_start(qsw[96:128, :], qraw[64:96, :])
                            # qro = qraw*cos + qsw*sin''   (sin sign pre-folded)
                            for ch in range(SC):
                                sl = slice(ch * 512, (ch + 1) * 512)
                                nc.gpsimd.tensor_tensor(
                                    qraw[:, sl], qraw[:, sl], cssb[:, 0, sl],
                                    AluOp.mult,
                                )
                                nc.gpsimd.tensor_tensor(
                                    qsw[:, sl], qsw[:, sl], cssb[:, 1, sl],
                                    AluOp.mult,
                                )
                                qsm = qk.tile(
                                    [P, 512], dt.float32, tag="qsm", bufs=3,
                                    name=f"qsm{t}{pr}{ch}",
                                )
                                nc.gpsimd.tensor_tensor(
                                    qsm[:], qraw[:, sl], qsw[:, sl], AluOp.add
                                )
                                qro = qk.tile(
                                    [P, 512], dt.float32r, tag="qro", bufs=3,
                                    name=f"qro{t}{pr}{ch}",
                                )
                                nc.vector.tensor_copy(qro[:], qsm[:])
                                qrl = qk.tile(
                                    [P, 512], dt.float32r, tag="qrl", bufs=3,
                                    name=f"qrl{t}{pr}{ch}",
                                )
                                nc.vector.tensor_tensor(
                                    qrl[:], qsm[:], qro[:], AluOp.subtract
                                )
                                nc.gpsimd.dma_start(
                                    qkrot[t, 0, 2 * pr : 2 * pr + 2, :, sl].rearrange(
                                        "h d s -> (h d) s"
                                    ),
                                    qro[:],
                                )
                                nc.gpsimd.dma_start(
                                    qkrot[t, 1, 2 * pr : 2 * pr + 2, :, sl].rearrange(
                                        "h d s -> (h d) s"
                                    ),
                                    qrl[:],
                                )

            # ---------------- phase 2: attention ----------------
            if BUILD_PHASES < 2:
                nc.compile()
                return nc
            with tc.tile_pool(name="at", bufs=1) as at:
                mdsb = at.tile([P, 4, 512], dt.float32r, tag="md", name="mdsb")
                nc.gpsimd.dma_start(mdsb[:], md[:])
                owsb = at.tile([P, 4, DM], dt.bfloat16, tag="ow", name="owsb")
                nc.gpsimd.dma_start(
                    owsb[:], ow.rearrange("(kt p) o -> p kt o", p=P)
                )
                oparts = [[None] * 2 for _ in range(ST)]
                for h in range(HPG):
                    kaug = at.tile(
                        [P, S], dt.float32r, tag="kaug", bufs=3, name=f"kaug{h}"
                    )
                    nc.gpsimd.dma_start(kaug[0:DK, :], qkrot[1, 0, h])
                    nc.gpsimd.memset(
                        kaug[DK : DK + 1, :].bitcast(dt.float32), 1.0
                    )
                    kaugl = at.tile(
                        [DK, S], dt.float32r, tag="kaugl", bufs=3, name=f"kaugl{h}"
                    )
                    nc.gpsimd.dma_start(kaugl[:], qkrot[1, 1, h])
                    for J in range(NJ):
                        qaug = at.tile(
                            [DK + 1, 512], dt.float32r, tag="qaug", bufs=3,
                            name=f"qaug{h}{J}",
                        )
                        nc.gpsimd.dma_start(
                            qaug[0:DK, :],
                            qkrot[0, 0, h, :, J * 512 : (J + 1) * 512],
                        )
                        qaugl = at.tile(
                            [DK, 512], dt.float32r, tag="qaugl", bufs=3,
                            name=f"qaugl{h}{J}",
                        )
                        nc.gpsimd.dma_start(
                            qaugl[:], qkrot[0, 1, h, :, J * 512 : (J + 1) * 512]
                        )
                        # pass 1: scores [q, kv] -> negated row max
                        negm4 = at.tile(
                            [P, 4], dt.float32r, tag="negm4", bufs=3,
                            name=f"negm4{h}{J}",
                        )
                        for qq in range(4):
                            qi = 4 * J + qq
                            kv = (qi + 1) * P
                            nch = (kv + 511) // 512
                            mparts = []
                            for c in range(nch):
                                cw = min(512, kv - c * 512)
                                sc = ps.tile(
                                    [P, 512], dt.float32, tag="sc", bufs=2,
                                    name=f"sc{h}{qi}{c}",
                                )
                                last_chunk = c == nch - 1
                                nc.tensor.matmul(
                                    sc[:, 0:cw],
                                    lhsT=qaug[0:DK, qq * P : (qq + 1) * P],
                                    rhs=kaug[0:DK, c * 512 : c * 512 + cw],
                                    start=True,
                                    stop=not last_chunk,
                                )
                                if last_chunk:
                                    doff = qi * P - c * 512
                                    nc.tensor.matmul(
                                        sc[:, doff : doff + P],
                                        lhsT=idrsb[:],
                                        rhs=mkmsb[:],
                                        start=False,
                                        stop=True,
                                    )
                                mp = at.tile(
                                    [P, 1], dt.float32r, tag="mp", bufs=8,
                                    name=f"mp{h}{qi}{c}",
                                )
                                nc.vector.reduce_max(
                                    mp[:], sc[:, 0:cw],
                                    axis=mybir.AxisListType.X,
                                    negate=True,
                                )
                                mparts.append(mp)
                            acc = mparts[0]
                            for m2 in mparts[1:]:
                                nc.vector.tensor_tensor(
                                    acc[:], acc[:], m2[:], AluOp.min
                                )
                            nc.vector.tensor_copy(negm4[:, qq : qq + 1], acc[:])
                        # negm4 [128,4] -> [1,512] into qaug row 64
                        ngt = ps.tile(
                            [4, P], dt.float32r, tag="av", bufs=4, name=f"ngt{h}{J}"
                        )
                        nc.tensor.transpose(ngt[:], negm4[:], idrsb[:])
                        ngs = at.tile(
                            [4, P], dt.float32r, tag="ngs", bufs=3, name=f"ngs{h}{J}"
                        )
                        nc.vector.tensor_copy(ngs[:], ngt[:])
                        for i4 in range(4):
                            nc.gpsimd.dma_start(
                                qaug[DK : DK + 1, i4 * P : (i4 + 1) * P],
                                ngs[i4 : i4 + 1, :],
                            )
                        # pass 2: scores^T [kv, q-512] (K=65 fold) -> exp -> attnT
                        atts = []
                        for j in range(4 * J + 4):
                            stp = ps.tile(
                                [P, 512], dt.float32, tag="pj", bufs=2,
                                name=f"stp{h}{J}{j}",
                            )
                            dj = j - 4 * J
                            nc.tensor.matmul(
                                stp[:],
                                lhsT=kaug[0 : DK + 1, j * P : (j + 1) * P],
                                rhs=qaug[0 : DK + 1, :],
                                start=True,
                                stop=False,
                            )
                            nc.tensor.matmul(
                                stp[:],
                                lhsT=kaug[0:DK, j * P : (j + 1) * P],
                                rhs=qaugl[:],
                                start=False,
                                stop=False,
                            )
                            nc.tensor.matmul(
                                stp[:],
                                lhsT=kaugl[:, j * P : (j + 1) * P],
                                rhs=qaug[0:DK, :],
                                start=False,
                                stop=(dj < 0),
                            )
                            if dj >= 0:
                                nc.tensor.matmul(
                                    stp[:],
                                    lhsT=idrsb[:],
                                    rhs=mdsb[:, dj, :],
                                    start=False,
                                    stop=True,
                                )
                            att = at.tile(
                                [P, 512], dt.bfloat16, tag="attnT", bufs=22,
                                name=f"att{h}{J}{j}",
                            )
                            nc.scalar.activation(att[:], stp[:], Act.Exp)
                            atts.append(att)
                        # AV with ones-augmented V; PSUM row 64 = denominator
                        avp = ps.tile(
                            [DK + 1, 512], dt.float32, tag="av", bufs=4,
                            name=f"avp{h}{J}",
                        )
                        nj = 4 * J + 4
                        for j in range(nj):
                            nc.tensor.matmul(
                                avp[:],
                                lhsT=vaug[:, j, h, :],
                                rhs=atts[j][:],
                                start=(j == 0),
                                stop=(j == nj - 1),
                            )
                        den = at.tile(
                            [DK + 1, 512], dt.float32, tag="den", bufs=3,
                            name=f"den{h}{J}",
                        )
                        nc.scalar.copy(den[DK : DK + 1, :], avp[DK : DK + 1, :])
                        nc.vector.reciprocal(
                            den[DK : DK + 1, :], den[DK : DK + 1, :]
                        )
                        denr = at.tile(
                            [DK + 1, 512], dt.float32r, tag="denr", bufs=3,
                            name=f"denr{h}{J}",
                        )
                        nc.vector.tensor_copy(
                            denr[DK : DK + 1, :], den[DK : DK + 1, :]
                        )
                        # replicate 1/den across partitions: rank-1 matmul
                        rk = ps.tile(
                            [DK, 512], dt.float32, tag="av", bufs=4,
                            name=f"rk{h}{J}",
                        )
                        nc.tensor.matmul(
                            rk[:],
                            lhsT=on1sb[DK : DK + 1, :],
                            rhs=denr[DK : DK + 1, :],
                            start=True,
                            stop=True,
                        )
                        bc = at.tile(
                            [DK, 512], dt.float32, tag="bc", bufs=3,
                            name=f"bc{h}{J}",
                        )
                        nc.scalar.copy(bc[:], rk[:])
                        pr, hh = h // 2, h % 2
                        if hh == 0:
                            dst = aoT[pr][0:DK, J * 512 : (J + 1) * 512]
                        else:
                            aotmp = at.tile(
                                [DK, S], dt.bfloat16, tag="aotmp", bufs=2,
                                name=f"aotmp{h}",
                            ) if J == 0 else aotmp
                            dst = aotmp[:, J * 512 : (J + 1) * 512]
                        nc.vector.tensor_tensor(
                            dst, avp[0:DK, :], bc[:], AluOp.mult
                        )
                    if h % 2 == 1:
                        # odd head -> partitions 64..127 of the pair tile
                        nc.gpsimd.dma_start(aoT[h // 2][DK:P, :], aotmp[:])

                # ---------------- output projection (tail) ----------------
                for st in range(ST):
                    for oc in range(2):
                        op_ = ps.tile(
                            [P, 512], dt.float32, tag="pj", bufs=2,
                            name=f"op{st}{oc}",
                        )
                        for pr4 in range(4):
                            nc.tensor.matmul(
                                op_[:],
                                lhsT=aoT[pr4][:, st * P : (st + 1) * P],
                                rhs=owsb[:, pr4, oc * 512 : (oc + 1) * 512],
                                start=(pr4 == 0),
                                stop=(pr4 == 3),
                            )
                        osb = at.tile(
                            [P, 512], dt.float32, tag="osb", bufs=3,
                            name=f"osb{st}{oc}",
                        )
                        nc.scalar.copy(osb[:], op_[:])
                        nc.gpsimd.dma_start(
                            opart[st * P : (st + 1) * P, oc * 512 : (oc + 1) * 512],
                            osb[:],
                        )

    nc.compile()
    return nc


def _rne11(x):
    """Replicate the PE's fp32r rounding: round-to-nearest-even, 11 mantissa bits."""
    b = x.view(np.uint32).astype(np.uint64)
    drop = np.uint64(12)
    half = np.uint64(1 << 11)
    low = b & np.uint64((1 << 12) - 1)
    base = b >> drop
    rup = (low > half) | ((low == half) & ((base & np.uint64(1)) == np.uint64(1)))
    return ((base + rup.astype(np.uint64)) << drop).astype(np.uint32).view(np.float32)


def _host_prep(x, q_w, k_w, v_w, o_w, S):
    """Build the 8 per-core input dicts (numpy arrays)."""
    import ml_dtypes

    perm = np.zeros(DM, dtype=np.int64)
    for h in range(H):
        for i in range(32):
            perm[h * DK + i] = h * DK + 2 * i
            perm[h * DK + 32 + i] = h * DK + 2 * i + 1
    q_wp = (q_w[perm] * 0.125).astype(np.float32)
    k_wp = k_w[perm].astype(np.float32)

    inv_freq = 1.0 / THETA ** (2.0 * np.arange(32, dtype=np.float64) / DK)
    pos = np.arange(S, dtype=np.float64)
    ang = inv_freq[:, None] * pos[None, :]          # [32, S]
    cos = np.cos(ang).astype(np.float32)
    sin = np.sin(ang).astype(np.float32)
    cs = np.zeros((P, 2, S), dtype=np.float32)
    for blk in range(2):
        b0 = blk * DK
        cs[b0 : b0 + 32, 0] = cos
        cs[b0 + 32 : b0 + 64, 0] = cos
        cs[b0 : b0 + 32, 1] = -sin
        cs[b0 + 32 : b0 + 64, 1] = sin

    idr = np.eye(P, dtype=np.float32)
    on1 = np.ones((P, DK), dtype=np.float32)
    r = np.arange(P)
    mkm = np.where(r[None, :] > r[:, None], np.float32(MASK_VAL), np.float32(0.0))
    md = np.zeros((P, 4, 512), dtype=np.float32)
    for dj in range(4):
        for qt in range(4):
            blkm = md[:, dj, qt * P : (qt + 1) * P]
            if qt < dj:
                blkm[:] = MASK_VAL
            elif qt == dj:
                blkm[:] = np.where(
                    r[None, :] < r[:, None], np.float32(MASK_VAL), np.float32(0.0)
                )

    in_maps = []
    for b in range(B):
        for g in range(NG):
            rows = slice(g * DG, (g + 1) * DG)
            xt = np.ascontiguousarray(x[b].T)
            xth = xt.astype(ml_dtypes.bfloat16)
            qkwf = np.ascontiguousarray(
                np.concatenate([q_wp[rows].T, k_wp[rows].T], axis=1)
            )
            qkwhh = qkwf.astype(ml_dtypes.bfloat16)
            in_maps.append(
                {
                    "xTh": xth,
                    "xTl": (xt - xth.astype(np.float32)).astype(ml_dtypes.bfloat16),
                    "qkwh": qkwhh,
                    "qkwl": (qkwf - qkwhh.astype(np.float32)).astype(
                        ml_dtypes.bfloat16
                    ),
                    "vw": np.ascontiguousarray(v_w[rows].T).astype(
                        ml_dtypes.bfloat16
                    ),
                    "ow": np.ascontiguousarray(o_w[:, rows].T).astype(
                        ml_dtypes.bfloat16
                    ),
                    "cs": cs,
                    "idr": idr,
                    "on1": on1,
                    "mkm": mkm,
                    "md": md,
                }
            )
    return in_maps


_NC_CACHE = {}


def kernel(x, q_w, k_w, v_w, o_w):
    import sys

    for p in ("/opt/trn_rl_repo",):
        if p not in sys.path:
            sys.path.insert(0, p)
    from concourse.bass_utils import run_bass_kernel_spmd

    x = np.asarray(x, dtype=np.float32)
    q_w = np.asarray(q_w, dtype=np.float32)
    k_w = np.asarray(k_w, dtype=np.float32)
    v_w = np.asarray(v_w, dtype=np.float32)
    o_w = np.asarray(o_w, dtype=np.float32)
    S = x.shape[1]

    if S not in _NC_CACHE:
        _NC_CACHE[S] = build_nc(S)
    nc = _NC_CACHE[S]

    in_maps = _host_prep(x, q_w, k_w, v_w, o_w, S)
    res = run_bass_kernel_spmd(nc, in_maps, core_ids=list(range(8)))

    out = np.zeros((B, S, DM), dtype=np.float32)
    for b in range(B):
        for g in range(NG):
            out[b] += res.results[b * NG + g]["opart"]
    return out

